# revision 7
# baseline (speedup 1.0000x reference)
"""Cost volume (tfa CorrelationCost, kernel_size=1, d=4) on 8 TRN2 cores.

out[b, k, y, x] = (1/C) * sum_c prv[b,c,y,x] * nxt_pad[b,c,y+dy,x+dx],
k = dy*9+dx, dy/dx in 0..8, nxt zero-padded by 4 on each spatial side.

Sharding: core i -> (batch b = i//2, H-half h = i%2). Each core gets the
full-C feature maps for its 64 rows (prv) and 72 padded rows (nxt).

Per-core algorithm (fp16 banded matmul), v5 -- latency/overlap-optimized
rewrite of v4 (93us). Three structural fixes over v4, driven by the trace:

1. v4 issued all 7 input DMAs up front across two queues; packet-level
   round-robin let every chunk share HBM bandwidth equally, so band-0's
   data landed only ~1us before ALL input data (first matmul at 25.4us).
   v5 puts every input DMA on the single gpsimd SWDGE queue in band
   order: in-order draining gives band 0 its 2.67MB at full line rate
   (~8us in), and compute overlaps the remaining input stream.

2. v4's evac (PSUM->SBUF fp32->fp16 cast) alternated whole bands between
   vector and scalar, one [128,384] op per tile: per-op fixed cost
   (120cyc DVE / 172cyc ACT) + sem made the steady-state tile pitch
   ~470ns and the compute phase 53us. v5 evacs PAIRS of tiles (two psum
   banks, [128,2,512] psum tiles; matmuls write cols 0:384 of each bank)
   in one op, alternating engines per pair: DVE (120+768)/0.96 ~ 925ns
   and ACT (172+768)/1.2 ~ 783ns per 2 tiles run concurrently -> ~245ns
   per tile.

3. v4's output used 8-partition stride-16 DMAs; 8 partitions map to only
   ~4 of 16 SBUF AXI ports, so the out phase ran at 120-210 GB/s and was
   an unoverlapped 22us tail. v5 reorders pixels q-major (partition
   m = 8q+r) so each band's entire slab dump is ONE 128-consecutive-
   partition DMA with a mixed partition+offset stride for the q dim
   (stride 8*ROW+512: 8 partitions down, 512 elems right -- the wy in
   [q, q+9) slab window), hitting all 16 ports at line rate. 4 output
   DMAs total, queued on the same SWDGE queue behind the inputs, so
   bands 0-2 drain during compute and only band 3 (1.18MB) is tail.

Traffic per core: prv 4.19MB + nxt 4.86MB + out 4.72MB = 13.8MB.
"""

import numpy as np

import bass_rust
import concourse.bass as bass
import concourse.tile as tile
from concourse import bacc, mybir
from concourse.bass_utils import run_bass_kernel_spmd

# Problem geometry (hardcoded per spec)
B, C, H, W = 4, 128, 128, 256
D = 4
ND = 2 * D + 1            # 9
K = ND * ND               # 81
HH = H // 2               # 64 rows per core
HP = HH + 2 * D           # 72 padded nxt rows per core
WP = W + 2 * D            # 264 padded nxt cols
YB, XB = 16, 8            # pixel tile: 16 rows x 8 cols = 128 partitions
NY, NX = YB + 2 * D, XB + 2 * D   # 24 x 16 window
NTY, NTX = HH // YB, W // XB      # 4 y-bands x 32 x-tiles
NWIN = NY * NX            # 384
N_CORES = 8

ROW = NTY * NY * NTX * NX         # 49152 stage elems per partition
BAND = NY * NTX * NX              # 12288
RUN = ND * NTX * NX               # 4608 (one slab run: 9 wy rows x 32 xb x 16 wx)

F16 = mybir.dt.float16
F32 = mybir.dt.float32

# Output DMA scheme: "mixed" = one 128-partition DMA per band using a
# mixed partition+offset stride for the q dim (exact 9-row slabs);
# "quad" = 4 DMAs per band over 32-consecutive-partition quads dumping
# 12 wy rows (host picks the 9 needed) -- fallback if mixed is rejected.
OUT_SCHEME = "quad"  # "mixed" is rejected by the BIR verifier (illegal partition step)


def build_nc():
    nc = bacc.Bacc("TRN2")
    prv_d = nc.declare_dram_parameter("prv_s", [C, NTY * NTX * 128], F16, isOutput=False)
    nxt_d = nc.declare_dram_parameter("nxt_s", [C, HP * WP], F16, isOutput=False)
    if OUT_SCHEME == "mixed":
        # out[q, r, band, run]: run = dy*512 + xb*16 + wx
        out_d = nc.declare_dram_parameter("out_s", [YB, XB, NTY, RUN], F16, isOutput=True)
    else:
        # out[band, quad, part-in-quad, 12*512]
        out_d = nc.declare_dram_parameter(
            "out_s", [NTY, 4, 32, 12 * NTX * NX], F16, isOutput=True
        )

    with tile.TileContext(nc) as tc:
        with (
            tc.tile_pool(name="inp", bufs=1) as inp,
            tc.tile_pool(name="psum", bufs=4, space="PSUM") as pp,
            tc.tile_pool(name="stage", bufs=1) as sp,
        ):
            prv_sb = inp.tile([C, NTY * NTX * 128], F16)
            nxt_sb = inp.tile([C, HP, WP], F16)
            # stage[part, yb, wy, xb, wx]: pixel (q, r) lives on partition
            # m = 8q + r; its 81 useful values are in wy rows [q, q+9).
            stage = sp.tile([128, NTY, NY, NTX, NX], F16)

            # All input DMAs on the single gpsimd SWDGE queue in band
            # order: single-queue FIFO draining means band 0's inputs get
            # the full HBM bandwidth and complete first (~8us), instead of
            # fair-sharing with every later chunk like v4.
            def nxt_chunk(j):  # 24-row chunks; band yb needs rows [16yb, 16yb+24)
                nc.gpsimd.dma_start(
                    nxt_sb[:, 24 * j : 24 * j + 24, :],
                    nxt_d[:, 24 * j * WP : (24 * j + 24) * WP],
                )

            def prv_chunk(lo_t, n_t):  # chunk of n_t tiles starting at tile lo_t
                lo = lo_t * 128
                nc.gpsimd.dma_start(
                    prv_sb[:, lo : lo + n_t * 128], prv_d[:, lo : lo + n_t * 128]
                )

            nxt_chunk(0)
            prv_chunk(0, 8)        # band 0 split so the first matmuls start early
            prv_chunk(8, 24)
            nxt_chunk(1)
            prv_chunk(32, 32)
            nxt_chunk(2)
            prv_chunk(64, 32)
            prv_chunk(96, 32)

            stage_t = stage[:, :, :, :, :].tensor

            for yb in range(NTY):
                # Absorb input-DMA waits on cheap PE instructions so each
                # matmul below carries only its psum-slot release wait.
                nc.tensor.ldweights(prv_sb[:, yb * NTX * 128 : yb * NTX * 128 + 1])
                nc.tensor.ldweights(nxt_sb[:, 16 * yb, :1])
                nc.tensor.ldweights(nxt_sb[:, 16 * yb + 23, :1])
                for xp in range(NTX // 2):
                    # One psum tile = 2 banks; matmul t writes bank t's
                    # cols [0, 384) -- both matmuls then evac'd in ONE op.
                    ps = pp.tile([128, 2, 512], F32)
                    for t in range(2):
                        xb = 2 * xp + t
                        ti = yb * NTX + xb
                        lhsT = prv_sb[:, ti * 128 : (ti + 1) * 128]
                        rhs = nxt_sb[:, yb * YB : yb * YB + NY, xb * XB : xb * XB + NX]
                        nc.tensor.matmul(ps[:, t, 0:NWIN], lhsT, rhs, start=True, stop=True)
                    # Evac both tiles in one op. Keep the PSUM source
                    # t-major (2 near-contiguous 1536B runs -- strided
                    # PSUM reads cost ~+24%/op, v5-measured) and put the
                    # (t, wy, wx) -> stage[wy, xb, wx] transpose on the
                    # SBUF write side, which is nearly free (v4 data).
                    src = ps[:, :, 0:NWIN]
                    dst = bass_rust.AP(
                        stage_t,
                        yb * BAND + 2 * xp * NX,
                        [[ROW, 128], [NX, 2], [NTX * NX, NY], [1, NX]],
                    )
                    pi = yb * (NTX // 2) + xp
                    if pi % 2 == 0:
                        nc.vector.tensor_copy(dst, src)
                    else:
                        nc.scalar.copy(dst, src)

            # Slab dump, one DMA per band (128 consecutive partitions ->
            # all 16 SBUF AXI ports -> line rate). Queued on the same
            # SWDGE queue behind the inputs: bands 0-2 drain during
            # compute; band 3 is the only tail.
            if OUT_SCHEME == "mixed":
                for b in range(NTY):
                    src = bass_rust.AP(
                        stage_t,
                        b * BAND,
                        [[8 * ROW + NTX * NX, YB], [ROW, XB], [1, RUN]],
                    )
                    nc.gpsimd.dma_start(out_d[:, :, b, :], src)
            else:
                # Quad i = partitions {i, i+4, ..., i+124} (stride 4): the
                # 32 partitions span all 16 SBUF AXI ports (consecutive-32
                # blocks only reach the 8 same-parity ports -> 216 GB/s,
                # v5-measured). Pixel (q, r) sits on partition
                # 32*(q%4) + 4*r + q//4, so quad i holds q in [4i, 4i+4)
                # whose slabs live in wy rows [4i, 4i+12).
                for b in range(NTY):
                    for i in range(4):
                        src = bass_rust.AP(
                            stage_t,
                            i * ROW + b * BAND + 4 * i * NTX * NX,
                            [[4 * ROW, 32], [1, 12 * NTX * NX]],
                        )
                        nc.gpsimd.dma_start(out_d[b, i], src)
    return nc


def make_in_maps(prv: np.ndarray, nxt: np.ndarray) -> list[dict[str, np.ndarray]]:
    prv = np.asarray(prv, dtype=np.float32)
    nxt = np.asarray(nxt, dtype=np.float32)
    nxt_pad = np.zeros((B, C, H + 2 * D, W + 2 * D), np.float32)
    nxt_pad[:, :, D : D + H, D : D + W] = nxt * np.float32(0.125)
    prv_s = prv * np.float32(0.0625)  # 2^-4 * 2^-3 = 1/C
    in_maps = []
    for core in range(N_CORES):
        b, h = divmod(core, 2)
        # prv tile-major, yb-outer; within a tile pixel (q, r) sits on
        # partition m = 32*(q%4) + 4*r + q//4 (port-spreading order for
        # the stride-4 quad out-DMAs): [C, yb, xb, q%4, r, q//4]
        p = prv_s[b, :, h * HH : (h + 1) * HH, :].reshape(C, NTY, 4, 4, NTX, XB)
        #                  axes: [C, yb, qh(4), ql(4), xb, r]
        p = np.ascontiguousarray(p.transpose(0, 1, 4, 3, 5, 2)).reshape(C, -1)
        # nxt unbanded: [C, 72, 264]
        x = nxt_pad[b, :, h * HH : h * HH + HP, :]
        in_maps.append(
            {
                "prv_s": p.astype(np.float16),
                "nxt_s": np.ascontiguousarray(x).reshape(C, -1).astype(np.float16),
            }
        )
    return in_maps


def extract_core(O: np.ndarray) -> np.ndarray:
    """Slab dump -> [K, HH, W] fp32.

    mixed: O[q, r, band, dy*512 + xb*16 + wx] holds psum col
    (q+dy)*16 + wx of pixel (y=16*band+q, x=8*xb+r); displacement
    k=(dy,dx) is at wx = r + dx.
    quad:  O[band, quad, p, j*512 + xb*16 + wx] with p = partition-32*quad
    = 8*(q-4*quad)+r ... j = wy - 4*quad, so dy = j - (q - 4*quad).
    """
    dy, dx = np.divmod(np.arange(K), ND)              # [81]
    r = np.arange(XB)
    if OUT_SCHEME == "mixed":
        A = np.asarray(O).astype(np.float32).reshape(YB, XB, NTY, ND, NTX, NX)
        G = A.transpose(2, 0, 4, 1, 3, 5)             # [band, q, xb, r, dy, wx]
        ridx = np.broadcast_to(r[None, :], (K, XB))   # [81, 8]
        jidx = np.broadcast_to(dy[:, None], (K, XB))  # [81, 8]
        wxidx = r[None, :] + dx[:, None]              # [81, 8]
        T = G[:, :, :, ridx, jidx, wxidx]             # [band, q, xb, 81, r]
        T = T.transpose(3, 0, 1, 2, 4)                # [81, band, q, xb, r]
        return T.reshape(K, HH, W)
    else:
        A = np.asarray(O).astype(np.float32).reshape(NTY, 4, 4, XB, 12, NTX, NX)
        # A[band, quad, qq, r, j, xb, wx]: pixel q = 4*quad + qq,
        # wy = 4*quad + j -> dy = j - qq
        G = A.transpose(0, 1, 2, 5, 3, 4, 6)          # [band, quad, qq, xb, r, j, wx]
        ridx = np.broadcast_to(r[None, :], (K, XB))
        wxidx = r[None, :] + dx[:, None]
        out = np.empty((NTY, 4, 4, NTX, K, XB), np.float32)
        for qq in range(4):
            jidx = np.broadcast_to(dy[:, None] + qq, (K, XB))
            out[:, :, qq] = G[:, :, qq][:, :, :, ridx, jidx, wxidx]
        T = out.transpose(4, 0, 1, 2, 3, 5)           # [81, band, quad, qq, xb, r]
        return T.reshape(K, HH, W)


def run(prv: np.ndarray, nxt: np.ndarray, trace: bool = False):
    nc = build_nc()
    nc.finalize()
    in_maps = make_in_maps(prv, nxt)
    res = run_bass_kernel_spmd(nc, in_maps, list(range(N_CORES)), trace=trace)
    out = np.empty((B, K, H, W), np.float32)
    for core in range(N_CORES):
        b, h = divmod(core, 2)
        out[b, :, h * HH : (h + 1) * HH, :] = extract_core(res.results[core]["out_s"])
    return out, res


def kernel(prv: np.ndarray, nxt: np.ndarray) -> np.ndarray:
    out, _ = run(prv, nxt, trace=False)
    return out


if __name__ == "__main__":
    rng = np.random.default_rng(0)
    prv = rng.standard_normal((B, C, H, W), dtype=np.float32)
    nxt = rng.standard_normal((B, C, H, W), dtype=np.float32)
    out = kernel(prv, nxt)
    print(out.shape, out.dtype)


# revision 13
# speedup vs baseline: 1.1747x; 1.1747x over previous
"""Cost volume (tfa CorrelationCost, kernel_size=1, d=4) on 8 TRN2 cores.

out[b, k, y, x] = (1/C) * sum_c prv[b,c,y,x] * nxt_pad[b,c,y+dy,x+dx],
k = dy*9+dx, dy/dx in 0..8, nxt zero-padded by 4 on each spatial side.

Sharding: core i -> (batch b = i//2, H-half h = i%2). Each core gets the
full-C feature maps for its 64 rows (prv) and 72 padded rows (nxt).

Per-core algorithm (fp16 banded matmul), v5 -- latency/overlap-optimized
rewrite of v4 (93us). Three structural fixes over v4, driven by the trace:

1. v4 issued all 7 input DMAs up front across two queues; packet-level
   round-robin let every chunk share HBM bandwidth equally, so band-0's
   data landed only ~1us before ALL input data (first matmul at 25.4us).
   v5 puts every input DMA on the single gpsimd SWDGE queue in band
   order: in-order draining gives band 0 its 2.67MB at full line rate
   (~8us in), and compute overlaps the remaining input stream.

2. v4's evac (PSUM->SBUF fp32->fp16 cast) alternated whole bands between
   vector and scalar, one [128,384] op per tile: per-op fixed cost
   (120cyc DVE / 172cyc ACT) + sem made the steady-state tile pitch
   ~470ns and the compute phase 53us. v5 evacs PAIRS of tiles (two psum
   banks, [128,2,512] psum tiles; matmuls write cols 0:384 of each bank)
   in one op, alternating engines per pair: DVE (120+768)/0.96 ~ 925ns
   and ACT (172+768)/1.2 ~ 783ns per 2 tiles run concurrently -> ~245ns
   per tile.

3. v4's output used 8-partition stride-16 DMAs; 8 partitions map to only
   ~4 of 16 SBUF AXI ports, so the out phase ran at 120-210 GB/s and was
   an unoverlapped 22us tail. v5 reorders pixels q-major (partition
   m = 8q+r) so each band's entire slab dump is ONE 128-consecutive-
   partition DMA with a mixed partition+offset stride for the q dim
   (stride 8*ROW+512: 8 partitions down, 512 elems right -- the wy in
   [q, q+9) slab window), hitting all 16 ports at line rate. 4 output
   DMAs total, queued on the same SWDGE queue behind the inputs, so
   bands 0-2 drain during compute and only band 3 (1.18MB) is tail.

Traffic per core: prv 4.19MB + nxt 4.86MB + out 4.72MB = 13.8MB.
"""

import numpy as np

import bass_rust
import concourse.bass as bass
import concourse.tile as tile
from concourse import bacc, mybir
from concourse.bass_utils import run_bass_kernel_spmd

# Problem geometry (hardcoded per spec)
B, C, H, W = 4, 128, 128, 256
D = 4
ND = 2 * D + 1            # 9
K = ND * ND               # 81
HH = H // 2               # 64 rows per core
HP = HH + 2 * D           # 72 padded nxt rows per core
WP = W + 2 * D            # 264 padded nxt cols
YB, XB = 16, 8            # pixel tile: 16 rows x 8 cols = 128 partitions
NY, NX = YB + 2 * D, XB + 2 * D   # 24 x 16 window
NTY, NTX = HH // YB, W // XB      # 4 y-bands x 32 x-tiles
NWIN = NY * NX            # 384
N_CORES = 8

ROW = NTY * NY * NTX * NX         # 49152 stage elems per partition
BAND = NY * NTX * NX              # 12288
RUN = ND * NTX * NX               # 4608 (one slab run: 9 wy rows x 32 xb x 16 wx)

F16 = mybir.dt.float16
F32 = mybir.dt.float32

# Output DMA scheme: "mixed" = one 128-partition DMA per band using a
# mixed partition+offset stride for the q dim (exact 9-row slabs);
# "quad" = 4 DMAs per band over 32-consecutive-partition quads dumping
# 12 wy rows (host picks the 9 needed) -- fallback if mixed is rejected.
OUT_SCHEME = "quad"  # "mixed" is rejected by the BIR verifier (illegal partition step)


def build_nc():
    nc = bacc.Bacc("TRN2")
    prv_d = nc.declare_dram_parameter("prv_s", [C, NTY * NTX * 128], F16, isOutput=False)
    nxt_d = nc.declare_dram_parameter("nxt_s", [C, HP * WP], F16, isOutput=False)
    if OUT_SCHEME == "mixed":
        # out[q, r, band, run]: run = dy*512 + xb*16 + wx
        out_d = nc.declare_dram_parameter("out_s", [YB, XB, NTY, RUN], F16, isOutput=True)
    else:
        # out[band, quad, part-in-quad, 12*512]
        out_d = nc.declare_dram_parameter(
            "out_s", [NTY, 4, 32, 12 * NTX * NX], F16, isOutput=True
        )

    with tile.TileContext(nc) as tc:
        with (
            tc.tile_pool(name="inp", bufs=1) as inp,
            tc.tile_pool(name="psum", bufs=4, space="PSUM") as pp,
            tc.tile_pool(name="stage", bufs=1) as sp,
        ):
            prv_sb = inp.tile([C, NTY * NTX * 128], F16)
            nxt_sb = inp.tile([C, HP, WP], F16)
            # stage[part, yb, xp, t, wy, wx] (pair-contiguous): the evac of
            # psum pair (t = 0, 1) is then a fully contiguous 768-elem
            # tile-slice write -- v5/v6 showed any 48-short-run side costs
            # +216ns/op, and raw-AP evac dsts break Tile's range tracking
            # (v6: every out-DMA serialized behind the last evac).
            stage = sp.tile([128, NTY, NTX // 2, 2, NY, NX], F16)

            # All input DMAs on the single gpsimd SWDGE queue in band
            # order: single-queue FIFO draining means band 0's inputs get
            # the full HBM bandwidth and complete first (~8us), instead of
            # fair-sharing with every later chunk like v4.
            def nxt_chunk(j):  # 24-row chunks; band yb needs rows [16yb, 16yb+24)
                nc.gpsimd.dma_start(
                    nxt_sb[:, 24 * j : 24 * j + 24, :],
                    nxt_d[:, 24 * j * WP : (24 * j + 24) * WP],
                )

            def prv_chunk(lo_t, n_t):  # chunk of n_t tiles starting at tile lo_t
                lo = lo_t * 128
                nc.gpsimd.dma_start(
                    prv_sb[:, lo : lo + n_t * 128], prv_d[:, lo : lo + n_t * 128]
                )

            nxt_chunk(0)
            prv_chunk(0, 8)        # band 0 split so the first matmuls start early
            prv_chunk(8, 24)
            nxt_chunk(1)
            prv_chunk(32, 32)
            nxt_chunk(2)
            prv_chunk(64, 32)
            prv_chunk(96, 32)

            stage_t = stage[:, :, :, :, :, :].tensor

            for yb in range(NTY):
                # Absorb input-DMA waits on cheap PE instructions so each
                # matmul below carries only its psum-slot release wait.
                nc.tensor.ldweights(prv_sb[:, yb * NTX * 128 : yb * NTX * 128 + 1])
                nc.tensor.ldweights(nxt_sb[:, 16 * yb, :1])
                nc.tensor.ldweights(nxt_sb[:, 16 * yb + 23, :1])
                for xp in range(NTX // 2):
                    # One psum tile = 2 banks; matmul t writes bank t's
                    # cols [0, 384) -- both matmuls then evac'd in ONE op.
                    ps = pp.tile([128, 2, 512], F32)
                    for t in range(2):
                        xb = 2 * xp + t
                        ti = yb * NTX + xb
                        lhsT = prv_sb[:, ti * 128 : (ti + 1) * 128]
                        rhs = nxt_sb[:, yb * YB : yb * YB + NY, xb * XB : xb * XB + NX]
                        nc.tensor.matmul(ps[:, t, 0:NWIN], lhsT, rhs, start=True, stop=True)
                    # Evac both tiles in one op: psum src is two 1536B
                    # runs, stage dst is one contiguous 768-elem run, and
                    # both are tile slices so Tile's range tracker keeps
                    # the per-band out-DMA deps precise.
                    src = ps[:, :, 0:NWIN]
                    dst = stage[:, yb, xp, :, :, :]
                    pi = yb * (NTX // 2) + xp
                    if pi % 2 == 0:
                        nc.vector.tensor_copy(dst, src)
                    else:
                        nc.scalar.copy(dst, src)

            # Slab dump, one DMA per band (128 consecutive partitions ->
            # all 16 SBUF AXI ports -> line rate). Queued on the same
            # SWDGE queue behind the inputs: bands 0-2 drain during
            # compute; band 3 is the only tail.
            if OUT_SCHEME == "mixed":
                for b in range(NTY):
                    src = bass_rust.AP(
                        stage_t,
                        b * BAND,
                        [[8 * ROW + NTX * NX, YB], [ROW, XB], [1, RUN]],
                    )
                    nc.gpsimd.dma_start(out_d[:, :, b, :], src)
            else:
                # Quad i = partitions {i, i+4, ..., i+124} (stride 4): the
                # 32 partitions span all 16 SBUF AXI ports (consecutive-32
                # blocks only reach the 8 same-parity ports -> 216 GB/s,
                # v5-measured). Pixel (q, r) sits on partition
                # 32*(q%4) + 4*r + q//4, so quad i holds q in [4i, 4i+4)
                # whose slabs live in wy rows [4i, 4i+12). In the pair-
                # contiguous stage layout the slab is a [xb(32), 192] AP
                # per partition (12 wy rows x 16 wx contiguous per xb).
                for b in range(NTY):
                    for i in range(4):
                        src = bass_rust.AP(
                            stage_t,
                            i * ROW + b * BAND + 4 * i * NX,
                            [[4 * ROW, 32], [NWIN, NTX], [1, 12 * NX]],
                        )
                        nc.gpsimd.dma_start(out_d[b, i], src)
    return nc


def make_in_maps(prv: np.ndarray, nxt: np.ndarray) -> list[dict[str, np.ndarray]]:
    prv = np.asarray(prv, dtype=np.float32)
    nxt = np.asarray(nxt, dtype=np.float32)
    nxt_pad = np.zeros((B, C, H + 2 * D, W + 2 * D), np.float32)
    nxt_pad[:, :, D : D + H, D : D + W] = nxt * np.float32(0.125)
    prv_s = prv * np.float32(0.0625)  # 2^-4 * 2^-3 = 1/C
    in_maps = []
    for core in range(N_CORES):
        b, h = divmod(core, 2)
        # prv tile-major, yb-outer; within a tile pixel (q, r) sits on
        # partition m = 32*(q%4) + 4*r + q//4 (port-spreading order for
        # the stride-4 quad out-DMAs): [C, yb, xb, q%4, r, q//4]
        p = prv_s[b, :, h * HH : (h + 1) * HH, :].reshape(C, NTY, 4, 4, NTX, XB)
        #                  axes: [C, yb, qh(4), ql(4), xb, r]
        p = np.ascontiguousarray(p.transpose(0, 1, 4, 3, 5, 2)).reshape(C, -1)
        # nxt unbanded: [C, 72, 264]
        x = nxt_pad[b, :, h * HH : h * HH + HP, :]
        in_maps.append(
            {
                "prv_s": p.astype(np.float16),
                "nxt_s": np.ascontiguousarray(x).reshape(C, -1).astype(np.float16),
            }
        )
    return in_maps


def extract_core(O: np.ndarray) -> np.ndarray:
    """Slab dump -> [K, HH, W] fp32.

    mixed: O[q, r, band, dy*512 + xb*16 + wx] holds psum col
    (q+dy)*16 + wx of pixel (y=16*band+q, x=8*xb+r); displacement
    k=(dy,dx) is at wx = r + dx.
    quad:  O[band, quad, s, xb*192 + j*16 + wx] with s = 8*(q-4*quad)+r
    (partition 4*s + quad), j = wy - 4*quad, so dy = j - (q - 4*quad).
    """
    dy, dx = np.divmod(np.arange(K), ND)              # [81]
    r = np.arange(XB)
    if OUT_SCHEME == "mixed":
        A = np.asarray(O).astype(np.float32).reshape(YB, XB, NTY, ND, NTX, NX)
        G = A.transpose(2, 0, 4, 1, 3, 5)             # [band, q, xb, r, dy, wx]
        ridx = np.broadcast_to(r[None, :], (K, XB))   # [81, 8]
        jidx = np.broadcast_to(dy[:, None], (K, XB))  # [81, 8]
        wxidx = r[None, :] + dx[:, None]              # [81, 8]
        T = G[:, :, :, ridx, jidx, wxidx]             # [band, q, xb, 81, r]
        T = T.transpose(3, 0, 1, 2, 4)                # [81, band, q, xb, r]
        return T.reshape(K, HH, W)
    else:
        A = np.asarray(O).astype(np.float32).reshape(NTY, 4, 4, XB, NTX, 12, NX)
        # A[band, quad, qq, r, xb, j, wx]: pixel q = 4*quad + qq,
        # wy = 4*quad + j -> dy = j - qq
        G = A.transpose(0, 1, 2, 4, 3, 5, 6)          # [band, quad, qq, xb, r, j, wx]
        ridx = np.broadcast_to(r[None, :], (K, XB))
        wxidx = r[None, :] + dx[:, None]
        out = np.empty((NTY, 4, 4, NTX, K, XB), np.float32)
        for qq in range(4):
            jidx = np.broadcast_to(dy[:, None] + qq, (K, XB))
            out[:, :, qq] = G[:, :, qq][:, :, :, ridx, jidx, wxidx]
        T = out.transpose(4, 0, 1, 2, 3, 5)           # [81, band, quad, qq, xb, r]
        return T.reshape(K, HH, W)


def run(prv: np.ndarray, nxt: np.ndarray, trace: bool = False):
    nc = build_nc()
    nc.finalize()
    in_maps = make_in_maps(prv, nxt)
    res = run_bass_kernel_spmd(nc, in_maps, list(range(N_CORES)), trace=trace)
    out = np.empty((B, K, H, W), np.float32)
    for core in range(N_CORES):
        b, h = divmod(core, 2)
        out[b, :, h * HH : (h + 1) * HH, :] = extract_core(res.results[core]["out_s"])
    return out, res


def kernel(prv: np.ndarray, nxt: np.ndarray) -> np.ndarray:
    out, _ = run(prv, nxt, trace=False)
    return out


if __name__ == "__main__":
    rng = np.random.default_rng(0)
    prv = rng.standard_normal((B, C, H, W), dtype=np.float32)
    nxt = rng.standard_normal((B, C, H, W), dtype=np.float32)
    out = kernel(prv, nxt)
    print(out.shape, out.dtype)


# revision 14
# speedup vs baseline: 1.1944x; 1.0167x over previous
"""Cost volume (tfa CorrelationCost, kernel_size=1, d=4) on 8 TRN2 cores.

out[b, k, y, x] = (1/C) * sum_c prv[b,c,y,x] * nxt_pad[b,c,y+dy,x+dx],
k = dy*9+dx, dy/dx in 0..8, nxt zero-padded by 4 on each spatial side.

Sharding: core i -> (batch b = i//2, H-half h = i%2). Each core gets the
full-C feature maps for its 64 rows (prv) and 72 padded rows (nxt).

Per-core algorithm (fp16 banded matmul), v8. Pipeline structure driven
by v4-v7 traces:

- All input DMAs ride the single gpsimd SWDGE queue in compute order;
  in-order draining gives the first tiles their data at full line rate
  instead of fair-sharing with later chunks (v4's first matmul waited
  25us; v8 starts ~12us).
- nxt is loaded in 4 column panels of 72 cols (8-col overlap, +9%
  bytes) x 3 row-chunks, so the first matmul needs only panel(0,0) +
  8 prv tiles = 0.5MB of input.
- Matmul pairs write [128, 2, 512] fp32 psum tiles (2 banks); one evac
  op (fp32->fp16 cast) moves both tiles, alternating vector/scalar per
  pair: measured ~1142/1081 ns per pair -> ~278ns/tile steady pitch
  with both PSUM readers saturated (this is the compute wall; per-op
  cost is AP-layout-invariant, v5/v6/v7-measured).
- stage[part, yb, wy, xb, wx] with pixel (q, r) on partition
  32*(q%4) + 4r + q//4. Output = 4 quad-DMAs per band: quad i reads
  partitions {i, i+4, ..., i+124} (stride 4 spans all 16 SBUF AXI
  ports; consecutive-32 blocks only reach 8 same-parity ports) and
  dumps wy rows [4i, 4i+12) as ONE contiguous 12.3KB run per partition
  (32 descriptors/DMA, line rate). Host picks the 9 needed rows.
- Out quads queue on the same SWDGE queue behind the inputs (bands 0-2
  drain during compute); band 3's four quads are spread across
  gpsimd/sync/scalar DGEs so their descriptor generation runs in
  parallel in the tail.

Traffic per core: prv 4.19MB + nxt 5.31MB + out 6.29MB = 15.8MB.
"""

import numpy as np

import bass_rust
import concourse.bass as bass
import concourse.tile as tile
from concourse import bacc, mybir
from concourse.bass_utils import run_bass_kernel_spmd

# Problem geometry (hardcoded per spec)
B, C, H, W = 4, 128, 128, 256
D = 4
ND = 2 * D + 1            # 9
K = ND * ND               # 81
HH = H // 2               # 64 rows per core
HP = HH + 2 * D           # 72 padded nxt rows per core
WP = W + 2 * D            # 264 padded nxt cols
YB, XB = 16, 8            # pixel tile: 16 rows x 8 cols = 128 partitions
NY, NX = YB + 2 * D, XB + 2 * D   # 24 x 16 window
NTY, NTX = HH // YB, W // XB      # 4 y-bands x 32 x-tiles
NWIN = NY * NX            # 384
N_CORES = 8
NP = 4                    # nxt column panels
PW = 72                   # panel width (64 + 8 halo)

ROW = NTY * NY * NTX * NX         # 49152 stage elems per partition
BAND = NY * NTX * NX              # 12288
QRUN = 12 * NTX * NX              # 6144 (quad slab: 12 wy rows x 32 xb x 16 wx)

F16 = mybir.dt.float16
F32 = mybir.dt.float32


def build_nc():
    nc = bacc.Bacc("TRN2")
    prv_d = nc.declare_dram_parameter("prv_s", [C, NTY * NTX * 128], F16, isOutput=False)
    nxt_d = nc.declare_dram_parameter("nxt_s", [C, NP * HP * PW], F16, isOutput=False)
    out_d = nc.declare_dram_parameter("out_s", [NTY, 4, 32, QRUN], F16, isOutput=True)

    with tile.TileContext(nc) as tc:
        with (
            tc.tile_pool(name="inp", bufs=1) as inp,
            tc.tile_pool(name="psum", bufs=4, space="PSUM") as pp,
            tc.tile_pool(name="stage", bufs=1) as sp,
        ):
            prv_sb = inp.tile([C, NTY * NTX * 128], F16)
            nxt_sb = inp.tile([C, NP, HP, PW], F16)
            # stage[part, yb, wy, xb, wx]: pixel (q, r) on partition
            # 32*(q%4) + 4r + q//4; its slab is wy rows [q, q+9).
            stage = sp.tile([128, NTY, NY, NTX, NX], F16)

            def nxt_chunk(j, p):  # rows [24j, 24j+24) of panel p
                lo = (p * HP + 24 * j) * PW
                nc.gpsimd.dma_start(
                    nxt_sb[:, p, 24 * j : 24 * j + 24, :],
                    nxt_d[:, lo : lo + 24 * PW],
                )

            def prv_chunk(lo_t, n_t):  # chunk of n_t tiles from tile lo_t
                lo = lo_t * 128
                nc.gpsimd.dma_start(
                    prv_sb[:, lo : lo + n_t * 128], prv_d[:, lo : lo + n_t * 128]
                )

            # Input order = compute order; single queue => in-order
            # completion at full bandwidth.
            nxt_chunk(0, 0); prv_chunk(0, 8)
            nxt_chunk(0, 1); prv_chunk(8, 8)
            nxt_chunk(0, 2); prv_chunk(16, 8)
            nxt_chunk(0, 3); prv_chunk(24, 8)
            nxt_chunk(1, 0); nxt_chunk(1, 1); prv_chunk(32, 16)
            nxt_chunk(1, 2); nxt_chunk(1, 3); prv_chunk(48, 16)
            nxt_chunk(2, 0); nxt_chunk(2, 1); prv_chunk(64, 16)
            nxt_chunk(2, 2); nxt_chunk(2, 3); prv_chunk(80, 16)
            prv_chunk(96, 16); prv_chunk(112, 16)

            stage_t = stage[:, :, :, :, :].tensor

            for yb in range(NTY):
                # Absorb band-level input waits on cheap PE instructions.
                nc.tensor.ldweights(prv_sb[:, yb * NTX * 128 : yb * NTX * 128 + 1])
                nc.tensor.ldweights(nxt_sb[:, 0, 16 * yb, :1])
                nc.tensor.ldweights(nxt_sb[:, NP - 1, 16 * yb + 23, :1])
                for xp in range(NTX // 2):
                    ps = pp.tile([128, 2, 512], F32)
                    for t in range(2):
                        xb = 2 * xp + t
                        ti = yb * NTX + xb
                        lhsT = prv_sb[:, ti * 128 : (ti + 1) * 128]
                        p, co = xb >> 3, 8 * (xb & 7)
                        rhs = nxt_sb[:, p, yb * YB : yb * YB + NY, co : co + NX]
                        nc.tensor.matmul(ps[:, t, 0:NWIN], lhsT, rhs, start=True, stop=True)
                    # One evac per pair; strided psum src costs the same
                    # as any other AP shape (fixed ~+216ns/op, measured),
                    # so keep the stage dst slab-friendly. dst is a tile
                    # slice: raw-AP WRITES break Tile's range tracking
                    # (v6: out-DMAs serialized behind the last evac).
                    src = bass_rust.AP(
                        ps[:, :, :].tensor,
                        0,
                        [[2 * 512, 128], [NX, NY], [512, 2], [1, NX]],
                    )
                    dst = stage[:, yb, :, 2 * xp : 2 * xp + 2, :]
                    pi = yb * (NTX // 2) + xp
                    if pi % 2 == 0:
                        nc.vector.tensor_copy(dst, src)
                    else:
                        nc.scalar.copy(dst, src)

            # Quad slab dump: quad i = partitions {i, i+4, ..., i+124},
            # one contiguous 6144-elem run per partition. Bands 0-2 on the
            # gpsimd queue (drain behind the inputs, during compute);
            # band 3 spread across three DGE engines for parallel issue.
            for b in range(NTY):
                engs = (
                    [nc.gpsimd] * 4
                    if b < NTY - 1
                    else [nc.gpsimd, nc.sync, nc.scalar, nc.sync]
                )
                for i in range(4):
                    src = bass_rust.AP(
                        stage_t,
                        i * ROW + b * BAND + 4 * i * NTX * NX,
                        [[4 * ROW, 32], [1, QRUN]],
                    )
                    engs[i].dma_start(out_d[b, i], src)
    return nc


def make_in_maps(prv: np.ndarray, nxt: np.ndarray) -> list[dict[str, np.ndarray]]:
    prv = np.asarray(prv, dtype=np.float32)
    nxt = np.asarray(nxt, dtype=np.float32)
    nxt_pad = np.zeros((B, C, H + 2 * D, W + 2 * D), np.float32)
    nxt_pad[:, :, D : D + H, D : D + W] = nxt * np.float32(0.125)
    prv_s = prv * np.float32(0.0625)  # 2^-4 * 2^-3 = 1/C
    in_maps = []
    for core in range(N_CORES):
        b, h = divmod(core, 2)
        # prv tile-major, yb-outer; within a tile pixel (q, r) sits on
        # partition m = 32*(q%4) + 4*r + q//4 (port-spreading order for
        # the stride-4 quad out-DMAs): [C, yb, xb, q%4, r, q//4]
        p = prv_s[b, :, h * HH : (h + 1) * HH, :].reshape(C, NTY, 4, 4, NTX, XB)
        #                  axes: [C, yb, qh(4), ql(4), xb, r]
        p = np.ascontiguousarray(p.transpose(0, 1, 4, 3, 5, 2)).reshape(C, -1)
        # nxt in 4 column panels of 72 (8-col overlap): [C, panel, 72, 72]
        x = nxt_pad[b, :, h * HH : h * HH + HP, :]
        xp = np.stack([x[:, :, 64 * q : 64 * q + PW] for q in range(NP)], axis=1)
        in_maps.append(
            {
                "prv_s": p.astype(np.float16),
                "nxt_s": np.ascontiguousarray(xp).reshape(C, -1).astype(np.float16),
            }
        )
    return in_maps


def extract_core(O: np.ndarray) -> np.ndarray:
    """Quad slab dump -> [K, HH, W] fp32.

    O[band, quad, s, j*512 + xb*16 + wx] with s = 8*(q-4*quad)+r
    (partition 4*s + quad), j = wy - 4*quad, so dy = j - (q - 4*quad);
    displacement k=(dy,dx) is at wx = r + dx.
    """
    dy, dx = np.divmod(np.arange(K), ND)              # [81]
    r = np.arange(XB)
    A = np.asarray(O).astype(np.float32).reshape(NTY, 4, 4, XB, 12, NTX, NX)
    # A[band, quad, qq, r, j, xb, wx]
    G = A.transpose(0, 1, 2, 5, 3, 4, 6)              # [band, quad, qq, xb, r, j, wx]
    ridx = np.broadcast_to(r[None, :], (K, XB))
    wxidx = r[None, :] + dx[:, None]
    out = np.empty((NTY, 4, 4, NTX, K, XB), np.float32)
    for qq in range(4):
        jidx = np.broadcast_to(dy[:, None] + qq, (K, XB))
        out[:, :, qq] = G[:, :, qq][:, :, :, ridx, jidx, wxidx]
    T = out.transpose(4, 0, 1, 2, 3, 5)               # [81, band, quad, qq, xb, r]
    return T.reshape(K, HH, W)


def run(prv: np.ndarray, nxt: np.ndarray, trace: bool = False):
    nc = build_nc()
    nc.finalize()
    in_maps = make_in_maps(prv, nxt)
    res = run_bass_kernel_spmd(nc, in_maps, list(range(N_CORES)), trace=trace)
    out = np.empty((B, K, H, W), np.float32)
    for core in range(N_CORES):
        b, h = divmod(core, 2)
        out[b, :, h * HH : (h + 1) * HH, :] = extract_core(res.results[core]["out_s"])
    return out, res


def kernel(prv: np.ndarray, nxt: np.ndarray) -> np.ndarray:
    out, _ = run(prv, nxt, trace=False)
    return out


if __name__ == "__main__":
    rng = np.random.default_rng(0)
    prv = rng.standard_normal((B, C, H, W), dtype=np.float32)
    nxt = rng.standard_normal((B, C, H, W), dtype=np.float32)
    out = kernel(prv, nxt)
    print(out.shape, out.dtype)


# revision 15
# speedup vs baseline: 1.2715x; 1.0645x over previous
"""Cost volume (tfa CorrelationCost, kernel_size=1, d=4) on 8 TRN2 cores.

out[b, k, y, x] = (1/C) * sum_c prv[b,c,y,x] * nxt_pad[b,c,y+dy,x+dx],
k = dy*9+dx, dy/dx in 0..8, nxt zero-padded by 4 on each spatial side.

Sharding: core i -> (batch b = i//2, H-half h = i%2). Each core gets the
full-C feature maps for its 64 rows (prv) and 72 padded rows (nxt).

Per-core algorithm (fp16 banded matmul), v8. Pipeline structure driven
by v4-v7 traces:

- All input DMAs ride the single gpsimd SWDGE queue in compute order;
  in-order draining gives the first tiles their data at full line rate
  instead of fair-sharing with later chunks (v4's first matmul waited
  25us; v8 starts ~12us).
- nxt is loaded in 4 column panels of 72 cols (8-col overlap, +9%
  bytes) x 3 row-chunks, so the first matmul needs only panel(0,0) +
  8 prv tiles = 0.5MB of input.
- Matmul pairs write [128, 2, 512] fp32 psum tiles (2 banks); one evac
  op (fp32->fp16 cast) moves both tiles, alternating vector/scalar per
  pair: measured ~1142/1081 ns per pair -> ~278ns/tile steady pitch
  with both PSUM readers saturated (this is the compute wall; per-op
  cost is AP-layout-invariant, v5/v6/v7-measured).
- stage[part, yb, wy, xb, wx] with pixel (q, r) on partition
  32*(q%4) + 4r + q//4. Output = 4 quad-DMAs per band: quad i reads
  partitions {i, i+4, ..., i+124} (stride 4 spans all 16 SBUF AXI
  ports; consecutive-32 blocks only reach 8 same-parity ports) and
  dumps wy rows [4i, 4i+12) as ONE contiguous 12.3KB run per partition
  (32 descriptors/DMA, line rate). Host picks the 9 needed rows.
- Out quads queue on the same SWDGE queue behind the inputs (bands 0-2
  drain during compute); band 3's four quads are spread across
  gpsimd/sync/scalar DGEs so their descriptor generation runs in
  parallel in the tail.

Traffic per core: prv 4.19MB + nxt 5.31MB + out 6.29MB = 15.8MB.
"""

import numpy as np

import bass_rust
import concourse.bass as bass
import concourse.tile as tile
from concourse import bacc, mybir
from concourse.bass_utils import run_bass_kernel_spmd

# Problem geometry (hardcoded per spec)
B, C, H, W = 4, 128, 128, 256
D = 4
ND = 2 * D + 1            # 9
K = ND * ND               # 81
HH = H // 2               # 64 rows per core
HP = HH + 2 * D           # 72 padded nxt rows per core
WP = W + 2 * D            # 264 padded nxt cols
YB, XB = 16, 8            # pixel tile: 16 rows x 8 cols = 128 partitions
NY, NX = YB + 2 * D, XB + 2 * D   # 24 x 16 window
NTY, NTX = HH // YB, W // XB      # 4 y-bands x 32 x-tiles
NWIN = NY * NX            # 384
N_CORES = 8
NP = 4                    # nxt column panels
PW = 72                   # panel width (64 + 8 halo)

ROW = NTY * NY * NTX * NX         # 49152 stage elems per partition
BAND = NY * NTX * NX              # 12288
QRUN = 12 * NTX * NX              # 6144 (quad slab: 12 wy rows x 32 xb x 16 wx)

F16 = mybir.dt.float16
F32 = mybir.dt.float32


def build_nc():
    nc = bacc.Bacc("TRN2")
    prv_d = nc.declare_dram_parameter("prv_s", [C, NTY * NTX * 128], F16, isOutput=False)
    nxt_d = nc.declare_dram_parameter("nxt_s", [C, NP * HP * PW], F16, isOutput=False)
    out_d = nc.declare_dram_parameter("out_s", [NTY, 4, 32, QRUN], F16, isOutput=True)

    with tile.TileContext(nc) as tc:
        with (
            tc.tile_pool(name="inp", bufs=1) as inp,
            tc.tile_pool(name="psum", bufs=4, space="PSUM") as pp,
            tc.tile_pool(name="stage", bufs=1) as sp,
        ):
            prv_sb = inp.tile([C, NTY * NTX * 128], F16)
            nxt_sb = inp.tile([C, NP, HP, PW], F16)
            # stage[part, yb, wy, xb, wx]: pixel (q, r) on partition
            # 32*(q%4) + 4r + q//4; its slab is wy rows [q, q+9).
            stage = sp.tile([128, NTY, NY, NTX, NX], F16)

            def nxt_chunk(j, p):  # rows [24j, 24j+24) of panel p
                lo = (p * HP + 24 * j) * PW
                nc.gpsimd.dma_start(
                    nxt_sb[:, p, 24 * j : 24 * j + 24, :],
                    nxt_d[:, lo : lo + 24 * PW],
                )

            def prv_chunk(lo_t, n_t):  # chunk of n_t tiles from tile lo_t
                lo = lo_t * 128
                nc.gpsimd.dma_start(
                    prv_sb[:, lo : lo + n_t * 128], prv_d[:, lo : lo + n_t * 128]
                )

            # Input order = compute order; single queue => in-order
            # completion at full bandwidth.
            nxt_chunk(0, 0); prv_chunk(0, 8)
            nxt_chunk(0, 1); prv_chunk(8, 8)
            nxt_chunk(0, 2); prv_chunk(16, 8)
            nxt_chunk(0, 3); prv_chunk(24, 8)
            nxt_chunk(1, 0); nxt_chunk(1, 1); prv_chunk(32, 16)
            nxt_chunk(1, 2); nxt_chunk(1, 3); prv_chunk(48, 16)
            nxt_chunk(2, 0); nxt_chunk(2, 1); prv_chunk(64, 16)
            nxt_chunk(2, 2); nxt_chunk(2, 3); prv_chunk(80, 16)
            prv_chunk(96, 16); prv_chunk(112, 16)

            stage_t = stage[:, :, :, :, :].tensor

            for yb in range(NTY):
                # Absorb band-level input waits on cheap PE instructions.
                nc.tensor.ldweights(prv_sb[:, yb * NTX * 128 : yb * NTX * 128 + 1])
                nc.tensor.ldweights(nxt_sb[:, 0, 16 * yb, :1])
                nc.tensor.ldweights(nxt_sb[:, NP - 1, 16 * yb + 23, :1])
                for xp in range(NTX // 2):
                    ps = pp.tile([128, 2, 512], F32)
                    for t in range(2):
                        xb = 2 * xp + t
                        ti = yb * NTX + xb
                        lhsT = prv_sb[:, ti * 128 : (ti + 1) * 128]
                        p, co = xb >> 3, 8 * (xb & 7)
                        rhs = nxt_sb[:, p, yb * YB : yb * YB + NY, co : co + NX]
                        nc.tensor.matmul(ps[:, t, 0:NWIN], lhsT, rhs, start=True, stop=True)
                    # One evac per pair; strided psum src costs the same
                    # as any other AP shape (fixed ~+216ns/op, measured),
                    # so keep the stage dst slab-friendly. dst is a tile
                    # slice: raw-AP WRITES break Tile's range tracking
                    # (v6: out-DMAs serialized behind the last evac).
                    src = bass_rust.AP(
                        ps[:, :, :].tensor,
                        0,
                        [[2 * 512, 128], [NX, NY], [512, 2], [1, NX]],
                    )
                    dst = stage[:, yb, :, 2 * xp : 2 * xp + 2, :]
                    pi = yb * (NTX // 2) + xp
                    if pi % 2 == 0:
                        nc.vector.tensor_copy(dst, src)
                    else:
                        nc.scalar.copy(dst, src)

            # Quad slab dump: quad i = partitions {i, i+4, ..., i+124},
            # one contiguous 6144-elem run per partition. Bands 0-2 on the
            # gpsimd queue (drain behind the inputs, during compute);
            # band 3 spread across three DGE engines for parallel issue.
            for b in range(NTY):
                engs = (
                    [nc.gpsimd] * 4
                    if b < NTY - 1
                    else [nc.gpsimd, nc.sync, nc.scalar, nc.sync]
                )
                for i in range(4):
                    # 3-dim AP form: the 2-dim [[4*ROW, 32], [1, 6144]]
                    # variant made Tile's tracker conservative (v8: every
                    # quad waited on ALL evacs, serializing the output
                    # stream after the last evac); this shape tracked
                    # per-band in v7. 512-elem runs stay at line rate.
                    src = bass_rust.AP(
                        stage_t,
                        i * ROW + b * BAND + 4 * i * NTX * NX,
                        [[4 * ROW, 32], [NTX * NX, 12], [1, NTX * NX]],
                    )
                    engs[i].dma_start(out_d[b, i], src)
    return nc


def make_in_maps(prv: np.ndarray, nxt: np.ndarray) -> list[dict[str, np.ndarray]]:
    prv = np.asarray(prv, dtype=np.float32)
    nxt = np.asarray(nxt, dtype=np.float32)
    nxt_pad = np.zeros((B, C, H + 2 * D, W + 2 * D), np.float32)
    nxt_pad[:, :, D : D + H, D : D + W] = nxt * np.float32(0.125)
    prv_s = prv * np.float32(0.0625)  # 2^-4 * 2^-3 = 1/C
    in_maps = []
    for core in range(N_CORES):
        b, h = divmod(core, 2)
        # prv tile-major, yb-outer; within a tile pixel (q, r) sits on
        # partition m = 32*(q%4) + 4*r + q//4 (port-spreading order for
        # the stride-4 quad out-DMAs): [C, yb, xb, q%4, r, q//4]
        p = prv_s[b, :, h * HH : (h + 1) * HH, :].reshape(C, NTY, 4, 4, NTX, XB)
        #                  axes: [C, yb, qh(4), ql(4), xb, r]
        p = np.ascontiguousarray(p.transpose(0, 1, 4, 3, 5, 2)).reshape(C, -1)
        # nxt in 4 column panels of 72 (8-col overlap): [C, panel, 72, 72]
        x = nxt_pad[b, :, h * HH : h * HH + HP, :]
        xp = np.stack([x[:, :, 64 * q : 64 * q + PW] for q in range(NP)], axis=1)
        in_maps.append(
            {
                "prv_s": p.astype(np.float16),
                "nxt_s": np.ascontiguousarray(xp).reshape(C, -1).astype(np.float16),
            }
        )
    return in_maps


def extract_core(O: np.ndarray) -> np.ndarray:
    """Quad slab dump -> [K, HH, W] fp32.

    O[band, quad, s, j*512 + xb*16 + wx] with s = 8*(q-4*quad)+r
    (partition 4*s + quad), j = wy - 4*quad, so dy = j - (q - 4*quad);
    displacement k=(dy,dx) is at wx = r + dx.
    """
    dy, dx = np.divmod(np.arange(K), ND)              # [81]
    r = np.arange(XB)
    A = np.asarray(O).astype(np.float32).reshape(NTY, 4, 4, XB, 12, NTX, NX)
    # A[band, quad, qq, r, j, xb, wx]
    G = A.transpose(0, 1, 2, 5, 3, 4, 6)              # [band, quad, qq, xb, r, j, wx]
    ridx = np.broadcast_to(r[None, :], (K, XB))
    wxidx = r[None, :] + dx[:, None]
    out = np.empty((NTY, 4, 4, NTX, K, XB), np.float32)
    for qq in range(4):
        jidx = np.broadcast_to(dy[:, None] + qq, (K, XB))
        out[:, :, qq] = G[:, :, qq][:, :, :, ridx, jidx, wxidx]
    T = out.transpose(4, 0, 1, 2, 3, 5)               # [81, band, quad, qq, xb, r]
    return T.reshape(K, HH, W)


def run(prv: np.ndarray, nxt: np.ndarray, trace: bool = False):
    nc = build_nc()
    nc.finalize()
    in_maps = make_in_maps(prv, nxt)
    res = run_bass_kernel_spmd(nc, in_maps, list(range(N_CORES)), trace=trace)
    out = np.empty((B, K, H, W), np.float32)
    for core in range(N_CORES):
        b, h = divmod(core, 2)
        out[b, :, h * HH : (h + 1) * HH, :] = extract_core(res.results[core]["out_s"])
    return out, res


def kernel(prv: np.ndarray, nxt: np.ndarray) -> np.ndarray:
    out, _ = run(prv, nxt, trace=False)
    return out


if __name__ == "__main__":
    rng = np.random.default_rng(0)
    prv = rng.standard_normal((B, C, H, W), dtype=np.float32)
    nxt = rng.standard_normal((B, C, H, W), dtype=np.float32)
    out = kernel(prv, nxt)
    print(out.shape, out.dtype)


# revision 17
# speedup vs baseline: 1.3153x; 1.0345x over previous
"""Cost volume (tfa CorrelationCost, kernel_size=1, d=4) on 8 TRN2 cores.

out[b, k, y, x] = (1/C) * sum_c prv[b,c,y,x] * nxt_pad[b,c,y+dy,x+dx],
k = dy*9+dx, dy/dx in 0..8, nxt zero-padded by 4 on each spatial side.

Sharding: core i -> (batch b = i//2, H-half h = i%2). Each core gets the
full-C feature maps for its 64 rows (prv) and 72 padded rows (nxt).

Per-core algorithm (fp16 banded matmul), v8. Pipeline structure driven
by v4-v7 traces:

- All input DMAs ride the single gpsimd SWDGE queue in compute order;
  in-order draining gives the first tiles their data at full line rate
  instead of fair-sharing with later chunks (v4's first matmul waited
  25us; v8 starts ~12us).
- nxt is loaded in 4 column panels of 72 cols (8-col overlap, +9%
  bytes) x 3 row-chunks, so the first matmul needs only panel(0,0) +
  8 prv tiles = 0.5MB of input.
- Matmul pairs write [128, 2, 512] fp32 psum tiles (2 banks); one evac
  op (fp32->fp16 cast) moves both tiles, alternating vector/scalar per
  pair: measured ~1142/1081 ns per pair -> ~278ns/tile steady pitch
  with both PSUM readers saturated (this is the compute wall; per-op
  cost is AP-layout-invariant, v5/v6/v7-measured).
- stage[part, yb, wy, xb, wx] with pixel (q, r) on partition
  32*(q%4) + 4r + q//4. Output = 4 quad-DMAs per band: quad i reads
  partitions {i, i+4, ..., i+124} (stride 4 spans all 16 SBUF AXI
  ports; consecutive-32 blocks only reach 8 same-parity ports) and
  dumps wy rows [4i, 4i+12) as ONE contiguous 12.3KB run per partition
  (32 descriptors/DMA, line rate). Host picks the 9 needed rows.
- Out quads queue on the same SWDGE queue behind the inputs (bands 0-2
  drain during compute); band 3's four quads are spread across
  gpsimd/sync/scalar DGEs so their descriptor generation runs in
  parallel in the tail.

Traffic per core: prv 4.19MB + nxt 5.31MB + out 6.29MB = 15.8MB.
"""

import numpy as np

import bass_rust
import concourse.bass as bass
import concourse.tile as tile
from concourse import bacc, mybir
from concourse.bass_utils import run_bass_kernel_spmd

# Problem geometry (hardcoded per spec)
B, C, H, W = 4, 128, 128, 256
D = 4
ND = 2 * D + 1            # 9
K = ND * ND               # 81
HH = H // 2               # 64 rows per core
HP = HH + 2 * D           # 72 padded nxt rows per core
WP = W + 2 * D            # 264 padded nxt cols
YB, XB = 16, 8            # pixel tile: 16 rows x 8 cols = 128 partitions
NY, NX = YB + 2 * D, XB + 2 * D   # 24 x 16 window
NTY, NTX = HH // YB, W // XB      # 4 y-bands x 32 x-tiles
NWIN = NY * NX            # 384
N_CORES = 8
NP = 4                    # nxt column panels
PW = 72                   # panel width (64 + 8 halo)

ROW = NTY * NY * NTX * NX         # 49152 stage elems per partition
BAND = NY * NTX * NX              # 12288
QRUN = 12 * NTX * NX              # 6144 (quad slab: 12 wy rows x 32 xb x 16 wx)

F16 = mybir.dt.float16
F32 = mybir.dt.float32


def build_nc():
    nc = bacc.Bacc("TRN2")
    prv_d = nc.declare_dram_parameter("prv_s", [C, NTY * NTX * 128], F16, isOutput=False)
    nxt_d = nc.declare_dram_parameter("nxt_s", [C, NP * HP * PW], F16, isOutput=False)
    out_d = nc.declare_dram_parameter("out_s", [NTY, 4, 32, QRUN], F16, isOutput=True)

    with tile.TileContext(nc) as tc:
        with (
            tc.tile_pool(name="inp", bufs=1) as inp,
            tc.tile_pool(name="psum", bufs=4, space="PSUM") as pp,
            tc.tile_pool(name="stage", bufs=1) as sp,
        ):
            prv_sb = inp.tile([C, NTY * NTX * 128], F16)
            nxt_sb = inp.tile([C, NP, HP, PW], F16)
            # stage[part, yb, wy, xb, wx]: pixel (q, r) on partition
            # 32*(q%4) + 4r + q//4; its slab is wy rows [q, q+9).
            stage = sp.tile([128, NTY, NY, NTX, NX], F16)

            def nxt_chunk(j, p):  # rows [24j, 24j+24) of panel p
                lo = (p * HP + 24 * j) * PW
                nc.gpsimd.dma_start(
                    nxt_sb[:, p, 24 * j : 24 * j + 24, :],
                    nxt_d[:, lo : lo + 24 * PW],
                )

            def prv_chunk(lo_t, n_t):  # chunk of n_t tiles from tile lo_t
                lo = lo_t * 128
                nc.gpsimd.dma_start(
                    prv_sb[:, lo : lo + n_t * 128], prv_d[:, lo : lo + n_t * 128]
                )

            # Input order = compute order; single queue => in-order
            # completion at full bandwidth.
            nxt_chunk(0, 0); prv_chunk(0, 4); prv_chunk(4, 4)
            nxt_chunk(0, 1); prv_chunk(8, 8)
            nxt_chunk(0, 2); prv_chunk(16, 8)
            nxt_chunk(0, 3); prv_chunk(24, 8)
            nxt_chunk(1, 0); nxt_chunk(1, 1); prv_chunk(32, 16)
            nxt_chunk(1, 2); nxt_chunk(1, 3); prv_chunk(48, 16)
            nxt_chunk(2, 0); nxt_chunk(2, 1); prv_chunk(64, 16)
            nxt_chunk(2, 2); nxt_chunk(2, 3); prv_chunk(80, 16)
            prv_chunk(96, 16); prv_chunk(112, 16)

            stage_t = stage[:, :, :, :, :].tensor

            for yb in range(NTY):
                # Absorb band-level input waits on cheap PE instructions.
                nc.tensor.ldweights(prv_sb[:, yb * NTX * 128 : yb * NTX * 128 + 1])
                nc.tensor.ldweights(nxt_sb[:, 0, 16 * yb, :1])
                nc.tensor.ldweights(nxt_sb[:, NP - 1, 16 * yb + 23, :1])
                for xp in range(NTX // 2):
                    ps = pp.tile([128, 2, 512], F32)
                    for t in range(2):
                        xb = 2 * xp + t
                        ti = yb * NTX + xb
                        lhsT = prv_sb[:, ti * 128 : (ti + 1) * 128]
                        p, co = xb >> 3, 8 * (xb & 7)
                        rhs = nxt_sb[:, p, yb * YB : yb * YB + NY, co : co + NX]
                        nc.tensor.matmul(ps[:, t, 0:NWIN], lhsT, rhs, start=True, stop=True)
                    # One evac per pair; strided psum src costs the same
                    # as any other AP shape (fixed ~+216ns/op, measured),
                    # so keep the stage dst slab-friendly. dst is a tile
                    # slice: raw-AP WRITES break Tile's range tracking
                    # (v6: out-DMAs serialized behind the last evac).
                    src = bass_rust.AP(
                        ps[:, :, :].tensor,
                        0,
                        [[2 * 512, 128], [NX, NY], [512, 2], [1, NX]],
                    )
                    dst = stage[:, yb, :, 2 * xp : 2 * xp + 2, :]
                    # 31/33 DVE/ACT split (ACT is ~6% faster per pair);
                    # the extra ACT pair sits mid-stream so the band-3
                    # finish stays balanced.
                    pi = yb * (NTX // 2) + xp
                    if pi % 2 == 0 and pi != 30:
                        nc.vector.tensor_copy(dst, src)
                    else:
                        nc.scalar.copy(dst, src)

            # Quad slab dump: quad i = partitions {i, i+4, ..., i+124},
            # one contiguous 6144-elem run per partition. Bands 0-2 on the
            # gpsimd queue (drain behind the inputs, during compute);
            # band 3 spread across three DGE engines for parallel issue.
            for b in range(NTY):
                engs = (
                    [nc.gpsimd] * 4
                    if b < NTY - 1
                    else [nc.gpsimd, nc.sync, nc.scalar, nc.sync]
                )
                for i in range(4):
                    # 3-dim AP form: the 2-dim [[4*ROW, 32], [1, 6144]]
                    # variant made Tile's tracker conservative (v8: every
                    # quad waited on ALL evacs, serializing the output
                    # stream after the last evac); this shape tracked
                    # per-band in v7. 512-elem runs stay at line rate.
                    src = bass_rust.AP(
                        stage_t,
                        i * ROW + b * BAND + 4 * i * NTX * NX,
                        [[4 * ROW, 32], [NTX * NX, 12], [1, NTX * NX]],
                    )
                    engs[i].dma_start(out_d[b, i], src)
    return nc


def make_in_maps(prv: np.ndarray, nxt: np.ndarray) -> list[dict[str, np.ndarray]]:
    prv = np.asarray(prv, dtype=np.float32)
    nxt = np.asarray(nxt, dtype=np.float32)
    nxt_pad = np.zeros((B, C, H + 2 * D, W + 2 * D), np.float32)
    nxt_pad[:, :, D : D + H, D : D + W] = nxt * np.float32(0.125)
    prv_s = prv * np.float32(0.0625)  # 2^-4 * 2^-3 = 1/C
    in_maps = []
    for core in range(N_CORES):
        b, h = divmod(core, 2)
        # prv tile-major, yb-outer; within a tile pixel (q, r) sits on
        # partition m = 32*(q%4) + 4*r + q//4 (port-spreading order for
        # the stride-4 quad out-DMAs): [C, yb, xb, q%4, r, q//4]
        p = prv_s[b, :, h * HH : (h + 1) * HH, :].reshape(C, NTY, 4, 4, NTX, XB)
        #                  axes: [C, yb, qh(4), ql(4), xb, r]
        p = np.ascontiguousarray(p.transpose(0, 1, 4, 3, 5, 2)).reshape(C, -1)
        # nxt in 4 column panels of 72 (8-col overlap): [C, panel, 72, 72]
        x = nxt_pad[b, :, h * HH : h * HH + HP, :]
        xp = np.stack([x[:, :, 64 * q : 64 * q + PW] for q in range(NP)], axis=1)
        in_maps.append(
            {
                "prv_s": p.astype(np.float16),
                "nxt_s": np.ascontiguousarray(xp).reshape(C, -1).astype(np.float16),
            }
        )
    return in_maps


def extract_core(O: np.ndarray) -> np.ndarray:
    """Quad slab dump -> [K, HH, W] fp32.

    O[band, quad, s, j*512 + xb*16 + wx] with s = 8*(q-4*quad)+r
    (partition 4*s + quad), j = wy - 4*quad, so dy = j - (q - 4*quad);
    displacement k=(dy,dx) is at wx = r + dx.
    """
    dy, dx = np.divmod(np.arange(K), ND)              # [81]
    r = np.arange(XB)
    A = np.asarray(O).astype(np.float32).reshape(NTY, 4, 4, XB, 12, NTX, NX)
    # A[band, quad, qq, r, j, xb, wx]
    G = A.transpose(0, 1, 2, 5, 3, 4, 6)              # [band, quad, qq, xb, r, j, wx]
    ridx = np.broadcast_to(r[None, :], (K, XB))
    wxidx = r[None, :] + dx[:, None]
    out = np.empty((NTY, 4, 4, NTX, K, XB), np.float32)
    for qq in range(4):
        jidx = np.broadcast_to(dy[:, None] + qq, (K, XB))
        out[:, :, qq] = G[:, :, qq][:, :, :, ridx, jidx, wxidx]
    T = out.transpose(4, 0, 1, 2, 3, 5)               # [81, band, quad, qq, xb, r]
    return T.reshape(K, HH, W)


def run(prv: np.ndarray, nxt: np.ndarray, trace: bool = False):
    nc = build_nc()
    nc.finalize()
    in_maps = make_in_maps(prv, nxt)
    res = run_bass_kernel_spmd(nc, in_maps, list(range(N_CORES)), trace=trace)
    out = np.empty((B, K, H, W), np.float32)
    for core in range(N_CORES):
        b, h = divmod(core, 2)
        out[b, :, h * HH : (h + 1) * HH, :] = extract_core(res.results[core]["out_s"])
    return out, res


def kernel(prv: np.ndarray, nxt: np.ndarray) -> np.ndarray:
    out, _ = run(prv, nxt, trace=False)
    return out


if __name__ == "__main__":
    rng = np.random.default_rng(0)
    prv = rng.standard_normal((B, C, H, W), dtype=np.float32)
    nxt = rng.standard_normal((B, C, H, W), dtype=np.float32)
    out = kernel(prv, nxt)
    print(out.shape, out.dtype)


# revision 19
# speedup vs baseline: 1.4516x; 1.1037x over previous
"""Cost volume (tfa CorrelationCost, kernel_size=1, d=4) on 8 TRN2 cores.

out[b, k, y, x] = (1/C) * sum_c prv[b,c,y,x] * nxt_pad[b,c,y+dy,x+dx],
k = dy*9+dx, dy/dx in 0..8, nxt zero-padded by 4 on each spatial side.

Sharding: core i -> (batch b = i//2, H-half h = i%2). Each core gets the
full-C feature maps for its 64 rows (prv) and 72 padded rows (nxt).

Per-core algorithm (fp16 banded matmul), v10 (59.3us vs 93.3us v4
baseline). Pipeline structure driven by v4-v9 traces:

- All input DMAs ride the single gpsimd SWDGE queue in compute order;
  in-order draining gives the first tiles their data at full line rate
  instead of fair-sharing with later chunks (v4's first matmul waited
  25us; v8 starts ~12us).
- nxt is loaded in 4 column panels of 72 cols (8-col overlap, +9%
  bytes) x 3 row-chunks, so the first matmul needs only panel(0,0) +
  8 prv tiles = 0.5MB of input.
- Matmul pairs write [128, 2, 512] fp32 psum tiles (2 banks); one evac
  op (fp32->fp16 cast) moves both tiles, alternating vector/scalar per
  pair (31/33 split, ACT is ~6% faster): measured ~1142/1081 ns per
  pair -> ~287ns/tile steady pitch with both PSUM readers saturated.
  This is the compute wall: fp32 PSUM sources cap both readers at 1
  elem/cyc (16-bit psum would unlock 2x but is TRN3-only), per-op cost
  is AP-layout-invariant (v5/v6/v7), and concurrent SDMA reads of the
  stage cost ~+20%/op in SBUF bank contention (v8 vs v9) but beat
  serializing the output stream by far.
- stage[part, yb, wy, xb, wx] with pixel (q, r) on partition
  32*(q%4) + 4r + q//4. Output = 4 quad-DMAs per band: quad i reads
  partitions {i, i+4, ..., i+124} (stride 4 spans all 16 SBUF AXI
  ports; consecutive-32 blocks only reach 8 same-parity ports) and
  dumps wy rows [4i, 4i+12) as ONE contiguous 12.3KB run per partition
  (32 descriptors/DMA, line rate). Host picks the 9 needed rows.
- Out quads queue on the same SWDGE queue behind the inputs (bands 0-2
  drain during compute); band 3's four quads are spread across
  gpsimd/sync/scalar DGEs so their descriptor generation runs in
  parallel in the tail.

Traffic per core: prv 4.19MB + nxt 5.31MB + out 6.29MB = 15.8MB.
"""

import numpy as np

import bass_rust
import concourse.bass as bass
import concourse.tile as tile
from concourse import bacc, mybir
from concourse.bass_utils import run_bass_kernel_spmd

# Problem geometry (hardcoded per spec)
B, C, H, W = 4, 128, 128, 256
D = 4
ND = 2 * D + 1            # 9
K = ND * ND               # 81
HH = H // 2               # 64 rows per core
HP = HH + 2 * D           # 72 padded nxt rows per core
WP = W + 2 * D            # 264 padded nxt cols
YB, XB = 16, 8            # pixel tile: 16 rows x 8 cols = 128 partitions
NY, NX = YB + 2 * D, XB + 2 * D   # 24 x 16 window
NTY, NTX = HH // YB, W // XB      # 4 y-bands x 32 x-tiles
NWIN = NY * NX            # 384
N_CORES = 8
NP = 4                    # nxt column panels
PW = 72                   # panel width (64 + 8 halo)

ROW = NTY * NY * NTX * NX         # 49152 stage elems per partition
BAND = NY * NTX * NX              # 12288
QRUN = 12 * NTX * NX              # 6144 (quad slab: 12 wy rows x 32 xb x 16 wx)

F16 = mybir.dt.float16
F32 = mybir.dt.float32


def build_nc():
    nc = bacc.Bacc("TRN2")
    prv_d = nc.declare_dram_parameter("prv_s", [C, NTY * NTX * 128], F16, isOutput=False)
    nxt_d = nc.declare_dram_parameter("nxt_s", [C, NP * HP * PW], F16, isOutput=False)
    out_d = nc.declare_dram_parameter("out_s", [NTY, 4, 32, QRUN], F16, isOutput=True)

    with tile.TileContext(nc) as tc:
        with (
            tc.tile_pool(name="inp", bufs=1) as inp,
            tc.tile_pool(name="psum", bufs=4, space="PSUM") as pp,
            tc.tile_pool(name="stage", bufs=1) as sp,
        ):
            prv_sb = inp.tile([C, NTY * NTX * 128], F16)
            nxt_sb = inp.tile([C, NP, HP, PW], F16)
            # stage[part, yb, wy, xb, wx]: pixel (q, r) on partition
            # 32*(q%4) + 4r + q//4; its slab is wy rows [q, q+9).
            stage = sp.tile([128, NTY, NY, NTX, NX], F16)

            def nxt_chunk(j, p):  # rows [24j, 24j+24) of panel p
                lo = (p * HP + 24 * j) * PW
                nc.gpsimd.dma_start(
                    nxt_sb[:, p, 24 * j : 24 * j + 24, :],
                    nxt_d[:, lo : lo + 24 * PW],
                )

            def prv_chunk(lo_t, n_t):  # chunk of n_t tiles from tile lo_t
                lo = lo_t * 128
                nc.gpsimd.dma_start(
                    prv_sb[:, lo : lo + n_t * 128], prv_d[:, lo : lo + n_t * 128]
                )

            # Input order = compute order; single queue => in-order
            # completion at full bandwidth.
            nxt_chunk(0, 0); prv_chunk(0, 4); prv_chunk(4, 4)
            nxt_chunk(0, 1); prv_chunk(8, 8)
            nxt_chunk(0, 2); prv_chunk(16, 8)
            nxt_chunk(0, 3); prv_chunk(24, 8)
            nxt_chunk(1, 0); nxt_chunk(1, 1); prv_chunk(32, 16)
            nxt_chunk(1, 2); nxt_chunk(1, 3); prv_chunk(48, 16)
            nxt_chunk(2, 0); nxt_chunk(2, 1); prv_chunk(64, 16)
            nxt_chunk(2, 2); nxt_chunk(2, 3); prv_chunk(80, 16)
            prv_chunk(96, 16); prv_chunk(112, 16)

            stage_t = stage[:, :, :, :, :].tensor

            for yb in range(NTY):
                # Absorb band-level input waits on cheap PE instructions.
                nc.tensor.ldweights(prv_sb[:, yb * NTX * 128 : yb * NTX * 128 + 1])
                nc.tensor.ldweights(nxt_sb[:, 0, 16 * yb, :1])
                nc.tensor.ldweights(nxt_sb[:, NP - 1, 16 * yb + 23, :1])
                for xp in range(NTX // 2):
                    ps = pp.tile([128, 2, 512], F32)
                    for t in range(2):
                        xb = 2 * xp + t
                        ti = yb * NTX + xb
                        lhsT = prv_sb[:, ti * 128 : (ti + 1) * 128]
                        p, co = xb >> 3, 8 * (xb & 7)
                        rhs = nxt_sb[:, p, yb * YB : yb * YB + NY, co : co + NX]
                        nc.tensor.matmul(ps[:, t, 0:NWIN], lhsT, rhs, start=True, stop=True)
                    # One evac per pair; strided psum src costs the same
                    # as any other AP shape (fixed ~+216ns/op, measured),
                    # so keep the stage dst slab-friendly. dst is a tile
                    # slice: raw-AP WRITES break Tile's range tracking
                    # (v6: out-DMAs serialized behind the last evac).
                    src = bass_rust.AP(
                        ps[:, :, :].tensor,
                        0,
                        [[2 * 512, 128], [NX, NY], [512, 2], [1, NX]],
                    )
                    dst = stage[:, yb, :, 2 * xp : 2 * xp + 2, :]
                    # 31/33 DVE/ACT split (ACT is ~6% faster per pair);
                    # the extra ACT pair sits mid-stream so the band-3
                    # finish stays balanced.
                    pi = yb * (NTX // 2) + xp
                    if pi % 2 == 0 and pi != 30:
                        nc.vector.tensor_copy(dst, src)
                    else:
                        nc.scalar.copy(dst, src)

            # Quad slab dump: quad i = partitions {i, i+4, ..., i+124},
            # one contiguous 6144-elem run per partition. Bands 0-2 on the
            # gpsimd queue (drain behind the inputs, during compute);
            # band 3 spread across three DGE engines for parallel issue.
            for b in range(NTY):
                engs = (
                    [nc.gpsimd] * 4
                    if b < NTY - 1
                    else [nc.gpsimd, nc.sync, nc.scalar, nc.sync]
                )
                for i in range(4):
                    # 3-dim AP form: the 2-dim [[4*ROW, 32], [1, 6144]]
                    # variant made Tile's tracker conservative (v8: every
                    # quad waited on ALL evacs, serializing the output
                    # stream after the last evac); this shape tracked
                    # per-band in v7. 512-elem runs stay at line rate.
                    src = bass_rust.AP(
                        stage_t,
                        i * ROW + b * BAND + 4 * i * NTX * NX,
                        [[4 * ROW, 32], [NTX * NX, 12], [1, NTX * NX]],
                    )
                    engs[i].dma_start(out_d[b, i], src)
    return nc


def make_in_maps(prv: np.ndarray, nxt: np.ndarray) -> list[dict[str, np.ndarray]]:
    prv = np.asarray(prv, dtype=np.float32)
    nxt = np.asarray(nxt, dtype=np.float32)
    nxt_pad = np.zeros((B, C, H + 2 * D, W + 2 * D), np.float32)
    nxt_pad[:, :, D : D + H, D : D + W] = nxt * np.float32(0.125)
    prv_s = prv * np.float32(0.0625)  # 2^-4 * 2^-3 = 1/C
    in_maps = []
    for core in range(N_CORES):
        b, h = divmod(core, 2)
        # prv tile-major, yb-outer; within a tile pixel (q, r) sits on
        # partition m = 32*(q%4) + 4*r + q//4 (port-spreading order for
        # the stride-4 quad out-DMAs): [C, yb, xb, q%4, r, q//4]
        p = prv_s[b, :, h * HH : (h + 1) * HH, :].reshape(C, NTY, 4, 4, NTX, XB)
        #                  axes: [C, yb, qh(4), ql(4), xb, r]
        p = np.ascontiguousarray(p.transpose(0, 1, 4, 3, 5, 2)).reshape(C, -1)
        # nxt in 4 column panels of 72 (8-col overlap): [C, panel, 72, 72]
        x = nxt_pad[b, :, h * HH : h * HH + HP, :]
        xp = np.stack([x[:, :, 64 * q : 64 * q + PW] for q in range(NP)], axis=1)
        in_maps.append(
            {
                "prv_s": p.astype(np.float16),
                "nxt_s": np.ascontiguousarray(xp).reshape(C, -1).astype(np.float16),
            }
        )
    return in_maps


def extract_core(O: np.ndarray) -> np.ndarray:
    """Quad slab dump -> [K, HH, W] fp32.

    O[band, quad, s, j*512 + xb*16 + wx] with s = 8*(q-4*quad)+r
    (partition 4*s + quad), j = wy - 4*quad, so dy = j - (q - 4*quad);
    displacement k=(dy,dx) is at wx = r + dx.
    """
    dy, dx = np.divmod(np.arange(K), ND)              # [81]
    r = np.arange(XB)
    A = np.asarray(O).astype(np.float32).reshape(NTY, 4, 4, XB, 12, NTX, NX)
    # A[band, quad, qq, r, j, xb, wx]
    G = A.transpose(0, 1, 2, 5, 3, 4, 6)              # [band, quad, qq, xb, r, j, wx]
    ridx = np.broadcast_to(r[None, :], (K, XB))
    wxidx = r[None, :] + dx[:, None]
    out = np.empty((NTY, 4, 4, NTX, K, XB), np.float32)
    for qq in range(4):
        jidx = np.broadcast_to(dy[:, None] + qq, (K, XB))
        out[:, :, qq] = G[:, :, qq][:, :, :, ridx, jidx, wxidx]
    T = out.transpose(4, 0, 1, 2, 3, 5)               # [81, band, quad, qq, xb, r]
    return T.reshape(K, HH, W)


def run(prv: np.ndarray, nxt: np.ndarray, trace: bool = False):
    nc = build_nc()
    nc.finalize()
    in_maps = make_in_maps(prv, nxt)
    res = run_bass_kernel_spmd(nc, in_maps, list(range(N_CORES)), trace=trace)
    out = np.empty((B, K, H, W), np.float32)
    for core in range(N_CORES):
        b, h = divmod(core, 2)
        out[b, :, h * HH : (h + 1) * HH, :] = extract_core(res.results[core]["out_s"])
    return out, res


def kernel(prv: np.ndarray, nxt: np.ndarray) -> np.ndarray:
    out, _ = run(prv, nxt, trace=False)
    return out


if __name__ == "__main__":
    rng = np.random.default_rng(0)
    prv = rng.standard_normal((B, C, H, W), dtype=np.float32)
    nxt = rng.standard_normal((B, C, H, W), dtype=np.float32)
    out = kernel(prv, nxt)
    print(out.shape, out.dtype)
